# revision 10
# baseline (speedup 1.0000x reference)
import sys

sys.path.insert(0, "/opt/trn_rl_repo")

import numpy as np
from contextlib import ExitStack

import concourse.bass as bass
import concourse.bacc as bacc
import concourse.tile as tile
from concourse import mybir
from concourse.bass_utils import run_bass_kernel_spmd
from concourse.masks import make_identity

B, C, H, W = 16, 64, 64, 64
HW = H * W          # 4096
M = HW // 4         # 1024
NCORES = 8
BPC = B // NCORES   # batches per core
F32 = mybir.dt.float32
BF16 = mybir.dt.bfloat16

NCHUNK = 512        # n-dim chunk (columns of s^T / o)
NCH = HW // NCHUNK  # 8 chunks per batch
MT = M // 128       # 8 m-tiles of 128


def _build_nc():
    nc = bacc.Bacc(None, target_bir_lowering=False)

    x_d = nc.dram_tensor("x", [BPC, C, HW], F32, kind="ExternalInput")
    wpgt_d = nc.dram_tensor("wpgt", [C, 40], F32, kind="ExternalInput")     # [w_g; w_phi]^T
    wtheta_d = nc.dram_tensor("wtheta", [8, C], F32, kind="ExternalInput")  # lhsT for G
    wot_d = nc.dram_tensor("wot", [32, C], F32, kind="ExternalInput")       # (gamma*w_o)^T
    out_d = nc.dram_tensor("out", [BPC, C, HW], F32, kind="ExternalOutput")
    srow_d = nc.dram_tensor("srow", [BPC, HW], F32)   # sumexp rows (internal)
    rd_d = nc.dram_tensor("rd", [BPC, HW], F32)       # recip rows (internal)

    with tile.TileContext(nc) as tc, ExitStack() as ctx:
        consts = ctx.enter_context(tc.tile_pool(name="consts", bufs=1))
        wpgt_sb = consts.tile([C, 40], F32)
        wtheta_sb = consts.tile([8, C], F32)
        wot_sb = consts.tile([32, C], F32)
        ident33 = consts.tile([33, 33], BF16)
        nc.sync.dma_start(out=wpgt_sb, in_=wpgt_d[:])
        nc.sync.dma_start(out=wtheta_sb, in_=wtheta_d[:])
        nc.sync.dma_start(out=wot_sb, in_=wot_d[:])
        make_identity(nc, ident33)

        # SBUF pools
        xp = ctx.enter_context(tc.tile_pool(name="xp", bufs=2))
        projp = ctx.enter_context(tc.tile_pool(name="projp", bufs=1))
        t1p = ctx.enter_context(tc.tile_pool(name="t1p", bufs=1))
        gtp = ctx.enter_context(tc.tile_pool(name="gtp", bufs=2))
        hbp = ctx.enter_context(tc.tile_pool(name="hbp", bufs=2))
        Gp = ctx.enter_context(tc.tile_pool(name="Gp", bufs=2))
        hTp = ctx.enter_context(tc.tile_pool(name="hTp", bufs=2))
        expp = ctx.enter_context(tc.tile_pool(name="expp", bufs=3))
        osbp = ctx.enter_context(tc.tile_pool(name="osbp", bufs=1))
        rbp = ctx.enter_context(tc.tile_pool(name="rbp", bufs=1))
        smallp = ctx.enter_context(tc.tile_pool(name="smallp", bufs=4))
        onp = ctx.enter_context(tc.tile_pool(name="onp", bufs=2))
        outp = ctx.enter_context(tc.tile_pool(name="outp", bufs=3))

        for b in range(BPC):
            x_sb = xp.tile([C, HW], F32)
            nc.sync.dma_start(out=x_sb, in_=x_d[b])

            # ---- phase A: projections, pooling, G, hT ----
            with tc.tile_pool(name="psA", bufs=2, space="PSUM") as psA:
                proj_sb = projp.tile([40, H, W], F32)
                for k in range(NCH):
                    pp = psA.tile([40, NCHUNK], F32)
                    nc.tensor.matmul(
                        pp, wpgt_sb, x_sb[:, k * NCHUNK:(k + 1) * NCHUNK],
                        start=True, stop=True,
                    )
                    nc.vector.tensor_copy(
                        proj_sb.rearrange("c h w -> c (h w)")[
                            :, k * NCHUNK:(k + 1) * NCHUNK
                        ],
                        pp,
                    )
                # maxpool 2x2: stage 1 over h pairs, stage 2 over w pairs
                t1 = t1p.tile([40, H // 2, W], F32)
                nc.vector.tensor_max(t1, proj_sb[:, 0::2, :], proj_sb[:, 1::2, :])
                g_t = gtp.tile([8, H // 2, W // 2], F32)
                nc.vector.tensor_max(g_t, t1[32:40, :, 0::2], t1[32:40, :, 1::2])
                hpb = hbp.tile([33, M], BF16)
                nc.vector.memset(hpb[32:33, :], 1.0)
                nc.vector.tensor_max(
                    hpb[0:32, :].rearrange("c (h w) -> c h w", h=H // 2),
                    t1[0:32, :, 0::2],
                    t1[0:32, :, 1::2],
                )

                # G = w_theta^T @ g  -> [64, M]
                Gps = psA.tile([C, M], F32)
                g_flat = g_t.rearrange("c h w -> c (h w)")
                for j in range(2):
                    nc.tensor.matmul(
                        Gps[:, j * 512:(j + 1) * 512],
                        wtheta_sb, g_flat[:, j * 512:(j + 1) * 512],
                        start=True, stop=True,
                    )
                G_sb = Gp.tile([C, M], F32)
                nc.vector.tensor_copy(G_sb, Gps)

                # hT: transpose h' [33, M] -> [128, MT, 34] (bf16, padded for align)
                ht_ps = psA.tile([128, MT, 34], BF16)
                for mt in range(MT):
                    nc.tensor.transpose(
                        ht_ps[:, mt, 0:33], hpb[:, mt * 128:(mt + 1) * 128], ident33
                    )
                hT_sb = hTp.tile([128, MT, 34], BF16)
                nc.vector.tensor_copy(hT_sb, ht_ps)

            # ---- phase B: attention ----
            o_sb = osbp.tile([33, HW], F32)
            with tc.tile_pool(name="psS", bufs=2, space="PSUM") as psS, \
                 tc.tile_pool(name="psO", bufs=2, space="PSUM") as psO:
                for k in range(NCH):
                    o_ps = psO.tile([33, NCHUNK], F32)
                    for mt in range(MT):
                        sT = psS.tile([128, NCHUNK], F32)
                        nc.tensor.matmul(
                            sT,
                            G_sb[:, mt * 128:(mt + 1) * 128],
                            x_sb[:, k * NCHUNK:(k + 1) * NCHUNK],
                            start=True, stop=True,
                        )
                        expT = expp.tile([128, NCHUNK], BF16)
                        nc.scalar.activation(
                            expT, sT, func=mybir.ActivationFunctionType.Exp
                        )
                        nc.tensor.matmul(
                            o_ps, hT_sb[:, mt, 0:33], expT,
                            start=(mt == 0), stop=(mt == MT - 1),
                        )
                    nc.vector.tensor_copy(
                        o_sb[:, k * NCHUNK:(k + 1) * NCHUNK], o_ps
                    )
                    nc.gpsimd.dma_start(
                        out=srow_d[b, k * NCHUNK:(k + 1) * NCHUNK],
                        in_=o_sb[32:33, k * NCHUNK:(k + 1) * NCHUNK],
                    )

            # ---- recip of sumexp, broadcast to 32 partitions ----
            rs = smallp.tile([128, HW // 128], F32)
            nc.sync.dma_start(
                out=rs, in_=srow_d[b].rearrange("(p i) -> p i", p=128)
            )
            rr = smallp.tile([128, HW // 128], F32)
            nc.vector.reciprocal(rr, rs)
            nc.sync.dma_start(
                out=rd_d[b].rearrange("(p i) -> p i", p=128), in_=rr
            )
            recipB = rbp.tile([32, HW], F32)
            rd_b = rd_d[b]
            nc.sync.dma_start(
                out=recipB,
                in_=bass.AP(
                    tensor=rd_b.tensor, offset=rd_b.offset,
                    ap=[[0, 32]] + list(rd_b.ap),
                ),
            )

            # ---- output: w_o @ (o * recip) * gamma + x ----
            with tc.tile_pool(name="psW", bufs=2, space="PSUM") as psW:
                for k in range(NCH):
                    ck = slice(k * NCHUNK, (k + 1) * NCHUNK)
                    onorm = onp.tile([32, NCHUNK], F32)
                    nc.vector.tensor_mul(onorm, o_sb[0:32, ck], recipB[:, ck])
                    wo_ps = psW.tile([C, NCHUNK], F32)
                    nc.tensor.matmul(wo_ps, wot_sb, onorm, start=True, stop=True)
                    outc = outp.tile([C, NCHUNK], F32)
                    nc.vector.tensor_add(outc, wo_ps, x_sb[:, ck])
                    nc.gpsimd.dma_start(out=out_d[b, :, ck], in_=outc)

    if not nc.is_finalized():
        nc.finalize()
    return nc


_NC_CACHE = {}


def _run(inputs: dict, trace: bool = False):
    if "nc" not in _NC_CACHE:
        _NC_CACHE["nc"] = _build_nc()
    nc = _NC_CACHE["nc"]

    x = np.ascontiguousarray(inputs["x"], dtype=np.float32).reshape(B, C, HW)
    wpgt = np.ascontiguousarray(
        np.concatenate([inputs["w_g"], inputs["w_phi"]], axis=0).T, dtype=np.float32
    )
    wtheta = np.ascontiguousarray(inputs["w_theta"], dtype=np.float32)
    wot = np.ascontiguousarray(
        (float(inputs["gamma"][0]) * inputs["w_o"]).T, dtype=np.float32
    )

    in_maps = []
    for i in range(NCORES):
        in_maps.append({
            "x": np.ascontiguousarray(x[i * BPC:(i + 1) * BPC]),
            "wpgt": wpgt,
            "wtheta": wtheta,
            "wot": wot,
        })

    res = run_bass_kernel_spmd(nc, in_maps, list(range(NCORES)), trace=trace)
    out = np.concatenate([r["out"] for r in res.results], axis=0)
    return out.reshape(B, C, H, W).astype(np.float32), res


def kernel(**inputs):
    out, _ = _run(inputs, trace=False)
    return out


# revision 13
# speedup vs baseline: 1.9088x; 1.9088x over previous
import sys

sys.path.insert(0, "/opt/trn_rl_repo")

import numpy as np
from contextlib import ExitStack

import concourse.bass as bass
import concourse.bacc as bacc
import concourse.tile as tile
from concourse import mybir
from concourse.bass_utils import run_bass_kernel_spmd
from concourse.masks import make_identity

B, C, H, W = 16, 64, 64, 64
HW = H * W          # 4096
M = HW // 4         # 1024
NCORES = 8
BPC = B // NCORES   # batches per core
F32 = mybir.dt.float32
BF16 = mybir.dt.bfloat16
FP8 = mybir.dt.float8e4

NCHUNK = 1024       # n-dim chunk (columns of s^T / o)
NCH = HW // NCHUNK  # 4 chunks per batch
MT = M // 128       # 8 m-tiles of 128


def _build_nc():
    nc = bacc.Bacc(None, target_bir_lowering=False)

    x_d = nc.dram_tensor("x", [BPC, C, HW], F32, kind="ExternalInput")
    wpgt_d = nc.dram_tensor("wpgt", [C, 40], BF16, kind="ExternalInput")     # [w_g; w_phi]^T
    wtheta_d = nc.dram_tensor("wtheta", [8, C], BF16, kind="ExternalInput")  # lhsT for G
    wot_d = nc.dram_tensor("wot", [32, C], BF16, kind="ExternalInput")       # (gamma*w_o)^T
    out_d = nc.dram_tensor("out", [BPC, C, HW], F32, kind="ExternalOutput")
    srow_d = nc.dram_tensor("srow", [BPC, HW], F32)   # sumexp rows (internal)
    rd_d = nc.dram_tensor("rd", [BPC, HW], F32)       # recip rows (internal)

    with tile.TileContext(nc) as tc, ExitStack() as ctx:
        consts = ctx.enter_context(tc.tile_pool(name="consts", bufs=1))
        wpgt_sb = consts.tile([C, 40], BF16)
        wtheta_sb = consts.tile([8, C], BF16)
        wot_sb = consts.tile([32, C], BF16)
        ident33 = consts.tile([33, 33], BF16)
        nc.sync.dma_start(out=wpgt_sb, in_=wpgt_d[:])
        nc.sync.dma_start(out=wtheta_sb, in_=wtheta_d[:])
        nc.sync.dma_start(out=wot_sb, in_=wot_d[:])
        make_identity(nc, ident33)

        # SBUF pools
        xp = ctx.enter_context(tc.tile_pool(name="xp", bufs=2))
        xbp = ctx.enter_context(tc.tile_pool(name="xbp", bufs=2))
        projp = ctx.enter_context(tc.tile_pool(name="projp", bufs=1))
        t1p = ctx.enter_context(tc.tile_pool(name="t1p", bufs=1))
        gtp = ctx.enter_context(tc.tile_pool(name="gtp", bufs=2))
        hbp = ctx.enter_context(tc.tile_pool(name="hbp", bufs=2))
        Gp = ctx.enter_context(tc.tile_pool(name="Gp", bufs=2))
        hTp = ctx.enter_context(tc.tile_pool(name="hTp", bufs=2))
        expp = ctx.enter_context(tc.tile_pool(name="expp", bufs=3))
        osbp = ctx.enter_context(tc.tile_pool(name="osbp", bufs=1))
        rbp = ctx.enter_context(tc.tile_pool(name="rbp", bufs=1))
        smallp = ctx.enter_context(tc.tile_pool(name="smallp", bufs=4))
        onp = ctx.enter_context(tc.tile_pool(name="onp", bufs=2))
        outp = ctx.enter_context(tc.tile_pool(name="outp", bufs=3))

        for b in range(BPC):
            x_sb = xp.tile([C, HW], F32)
            nc.sync.dma_start(out=x_sb, in_=x_d[b])
            xb_sb = xbp.tile([C, HW], BF16)
            nc.gpsimd.tensor_copy(xb_sb, x_sb)

            # ---- phase A: projections, pooling, G, hT ----
            with tc.tile_pool(name="psA", bufs=2, space="PSUM") as psA, \
                 tc.tile_pool(name="psG", bufs=1, space="PSUM") as psG, \
                 tc.tile_pool(name="psH", bufs=1, space="PSUM") as psH:
                proj_sb = projp.tile([40, H, W], BF16)
                for k in range(NCH):
                    pp = psA.tile([40, NCHUNK], F32)
                    for j in range(2):
                        sl = slice(k * NCHUNK + j * 512, k * NCHUNK + (j + 1) * 512)
                        nc.tensor.matmul(
                            pp[:, j * 512:(j + 1) * 512], wpgt_sb, xb_sb[:, sl],
                            start=True, stop=True,
                        )
                    nc.vector.tensor_copy(
                        proj_sb.rearrange("c h w -> c (h w)")[
                            :, k * NCHUNK:(k + 1) * NCHUNK
                        ],
                        pp,
                    )
                # maxpool 2x2: stage 1 over h pairs, stage 2 over w pairs
                t1 = t1p.tile([40, H // 2, W], BF16)
                nc.vector.tensor_max(t1, proj_sb[:, 0::2, :], proj_sb[:, 1::2, :])
                g_t = gtp.tile([8, H // 2, W // 2], BF16)
                nc.vector.tensor_max(g_t, t1[32:40, :, 0::2], t1[32:40, :, 1::2])
                hpb = hbp.tile([33, M], BF16)
                nc.vector.memset(hpb[32:33, :], 1.0)
                nc.vector.tensor_max(
                    hpb[0:32, :].rearrange("c (h w) -> c h w", h=H // 2),
                    t1[0:32, :, 0::2],
                    t1[0:32, :, 1::2],
                )

                # G = w_theta^T @ g  -> [64, M]
                Gps = psG.tile([C, M], F32)
                g_flat = g_t.rearrange("c h w -> c (h w)")
                for j in range(2):
                    nc.tensor.matmul(
                        Gps[:, j * 512:(j + 1) * 512],
                        wtheta_sb, g_flat[:, j * 512:(j + 1) * 512],
                        start=True, stop=True,
                    )
                G_sb = Gp.tile([C, M], BF16)
                nc.vector.tensor_copy(G_sb, Gps)

                # hT: transpose h' [33, M] -> [128, MT, 34] (bf16, padded for align)
                ht_ps = psH.tile([128, MT, 34], BF16)
                for mt in range(MT):
                    nc.tensor.transpose(
                        ht_ps[:, mt, 0:33], hpb[:, mt * 128:(mt + 1) * 128], ident33
                    )
                hT_sb = hTp.tile([128, MT, 34], BF16)
                nc.vector.tensor_copy(hT_sb, ht_ps)

            # ---- phase B: attention ----
            o_sb = osbp.tile([33, HW], F32)
            with tc.tile_pool(name="psS", bufs=2, space="PSUM") as psS, \
                 tc.tile_pool(name="psO", bufs=2, space="PSUM") as psO:
                for k in range(NCH):
                    o_ps = psO.tile([33, NCHUNK], F32)
                    for mt in range(MT):
                        sT = psS.tile([128, NCHUNK], F32)
                        for j in range(2):
                            sl = slice(
                                k * NCHUNK + j * 512, k * NCHUNK + (j + 1) * 512
                            )
                            nc.tensor.matmul(
                                sT[:, j * 512:(j + 1) * 512],
                                G_sb[:, mt * 128:(mt + 1) * 128],
                                xb_sb[:, sl],
                                start=True, stop=True,
                            )
                        expT = expp.tile([128, NCHUNK], FP8)
                        nc.scalar.activation(
                            expT, sT, func=mybir.ActivationFunctionType.Exp
                        )
                        for j in range(2):
                            nc.tensor.matmul(
                                o_ps[:, j * 512:(j + 1) * 512],
                                hT_sb[:, mt, 0:33],
                                expT[:, j * 512:(j + 1) * 512],
                                start=(mt == 0), stop=(mt == MT - 1),
                            )
                    nc.vector.tensor_copy(
                        o_sb[:, k * NCHUNK:(k + 1) * NCHUNK], o_ps
                    )
                    nc.gpsimd.dma_start(
                        out=srow_d[b, k * NCHUNK:(k + 1) * NCHUNK],
                        in_=o_sb[32:33, k * NCHUNK:(k + 1) * NCHUNK],
                    )

            # ---- recip of sumexp, broadcast to 32 partitions ----
            rs = smallp.tile([128, HW // 128], F32)
            nc.sync.dma_start(
                out=rs, in_=srow_d[b].rearrange("(p i) -> p i", p=128)
            )
            rr = smallp.tile([128, HW // 128], F32)
            nc.vector.reciprocal(rr, rs)
            nc.sync.dma_start(
                out=rd_d[b].rearrange("(p i) -> p i", p=128), in_=rr
            )
            recipB = rbp.tile([32, HW], F32)
            rd_b = rd_d[b]
            nc.sync.dma_start(
                out=recipB,
                in_=bass.AP(
                    tensor=rd_b.tensor, offset=rd_b.offset,
                    ap=[[0, 32]] + list(rd_b.ap),
                ),
            )

            # ---- output: w_o @ (o * recip) * gamma + x ----
            with tc.tile_pool(name="psW", bufs=2, space="PSUM") as psW:
                for k in range(NCH):
                    ck = slice(k * NCHUNK, (k + 1) * NCHUNK)
                    onorm = onp.tile([32, NCHUNK], BF16)
                    nc.vector.tensor_mul(onorm, o_sb[0:32, ck], recipB[:, ck])
                    wo_ps = psW.tile([C, NCHUNK], F32)
                    for j in range(2):
                        nc.tensor.matmul(
                            wo_ps[:, j * 512:(j + 1) * 512], wot_sb,
                            onorm[:, j * 512:(j + 1) * 512],
                            start=True, stop=True,
                        )
                    outc = outp.tile([C, NCHUNK], F32)
                    nc.vector.tensor_add(outc, wo_ps, x_sb[:, ck])
                    nc.gpsimd.dma_start(out=out_d[b, :, ck], in_=outc)

    if not nc.is_finalized():
        nc.finalize()
    return nc


_NC_CACHE = {}


def _run(inputs: dict, trace: bool = False):
    if "nc" not in _NC_CACHE:
        _NC_CACHE["nc"] = _build_nc()
    nc = _NC_CACHE["nc"]

    import ml_dtypes

    x = np.ascontiguousarray(inputs["x"], dtype=np.float32).reshape(B, C, HW)
    wpgt = np.ascontiguousarray(
        np.concatenate([inputs["w_g"], inputs["w_phi"]], axis=0).T.astype(
            ml_dtypes.bfloat16
        )
    )
    wtheta = np.ascontiguousarray(
        np.asarray(inputs["w_theta"]).astype(ml_dtypes.bfloat16)
    )
    wot = np.ascontiguousarray(
        (float(inputs["gamma"][0]) * inputs["w_o"]).T.astype(ml_dtypes.bfloat16)
    )

    in_maps = []
    for i in range(NCORES):
        in_maps.append({
            "x": np.ascontiguousarray(x[i * BPC:(i + 1) * BPC]),
            "wpgt": wpgt,
            "wtheta": wtheta,
            "wot": wot,
        })

    res = run_bass_kernel_spmd(nc, in_maps, list(range(NCORES)), trace=trace)
    out = np.concatenate([r["out"] for r in res.results], axis=0)
    return out.reshape(B, C, H, W).astype(np.float32), res


def kernel(**inputs):
    out, _ = _run(inputs, trace=False)
    return out


# revision 20
# speedup vs baseline: 2.3066x; 1.2084x over previous
import sys

sys.path.insert(0, "/opt/trn_rl_repo")

import numpy as np
from contextlib import ExitStack

import concourse.bass as bass
import concourse.bacc as bacc
import concourse.tile as tile
from concourse import mybir
from concourse.bass_utils import run_bass_kernel_spmd
from concourse.masks import make_identity

B, C, H, W = 16, 64, 64, 64
HW = H * W          # 4096
M = HW // 4         # 1024
NCORES = 8
BPC = B // NCORES   # batches per core
F32 = mybir.dt.float32
BF16 = mybir.dt.bfloat16
FP8 = mybir.dt.float8e4

NCHUNK = 1024       # n-dim chunk (columns of s^T / o)
NCH = HW // NCHUNK  # 4 chunks per batch
MT = M // 128       # 8 m-tiles of 128


def _build_nc():
    nc = bacc.Bacc(None, target_bir_lowering=False)

    x_d = nc.dram_tensor("x", [BPC, C, HW], F32, kind="ExternalInput")
    xb_d = nc.dram_tensor("xb", [BPC, C, HW], BF16, kind="ExternalInput")
    wpgt_d = nc.dram_tensor("wpgt", [C, 40], BF16, kind="ExternalInput")     # [w_g; w_phi]^T
    wtheta_d = nc.dram_tensor("wtheta", [8, C], BF16, kind="ExternalInput")  # lhsT for G
    wot_d = nc.dram_tensor("wot", [32, C], BF16, kind="ExternalInput")       # (gamma*w_o)^T
    out_d = nc.dram_tensor("out", [BPC, C, HW], F32, kind="ExternalOutput")
    srow_d = nc.dram_tensor("srow", [BPC, HW], F32)   # sumexp rows (internal)
    rd_d = nc.dram_tensor("rd", [BPC, HW], F32)       # recip rows (internal)

    with tile.TileContext(nc) as tc, ExitStack() as ctx:
        consts = ctx.enter_context(tc.tile_pool(name="consts", bufs=1))
        wpgt_sb = consts.tile([C, 40], BF16)
        wtheta_sb = consts.tile([8, C], BF16)
        wot_sb = consts.tile([32, C], BF16)
        ident33 = consts.tile([33, 33], BF16)
        nc.sync.dma_start(out=wpgt_sb, in_=wpgt_d[:])
        nc.sync.dma_start(out=wtheta_sb, in_=wtheta_d[:])
        nc.sync.dma_start(out=wot_sb, in_=wot_d[:])
        make_identity(nc, ident33)

        # SBUF pools
        xp = ctx.enter_context(tc.tile_pool(name="xp", bufs=2))
        xbp = ctx.enter_context(tc.tile_pool(name="xbp", bufs=2))
        projp = ctx.enter_context(tc.tile_pool(name="projp", bufs=1))
        t1p = ctx.enter_context(tc.tile_pool(name="t1p", bufs=1))
        gtp = ctx.enter_context(tc.tile_pool(name="gtp", bufs=2))
        hbp = ctx.enter_context(tc.tile_pool(name="hbp", bufs=2))
        Gp = ctx.enter_context(tc.tile_pool(name="Gp", bufs=2))
        hTp = ctx.enter_context(tc.tile_pool(name="hTp", bufs=2))
        expp = ctx.enter_context(tc.tile_pool(name="expp", bufs=3))
        osbp = ctx.enter_context(tc.tile_pool(name="osbp", bufs=1))
        rbp = ctx.enter_context(tc.tile_pool(name="rbp", bufs=1))
        smallp = ctx.enter_context(tc.tile_pool(name="smallp", bufs=4))
        onp = ctx.enter_context(tc.tile_pool(name="onp", bufs=2))
        outp = ctx.enter_context(tc.tile_pool(name="outp", bufs=3))

        for b in range(BPC):
            x_sb = xp.tile([C, HW], F32)
            nc.sync.dma_start(out=x_sb, in_=x_d[b])
            xb_sb = xbp.tile([C, HW], BF16)
            nc.sync.dma_start(out=xb_sb, in_=xb_d[b])

            # ---- phase A: projections, pooling, G, hT ----
            with tc.tile_pool(name="psA", bufs=2, space="PSUM") as psA, \
                 tc.tile_pool(name="psG", bufs=1, space="PSUM") as psG, \
                 tc.tile_pool(name="psH", bufs=1, space="PSUM") as psH:
                proj_sb = projp.tile([40, H, W], BF16)
                for k in range(NCH):
                    pp = psA.tile([40, NCHUNK], F32)
                    for j in range(2):
                        sl = slice(k * NCHUNK + j * 512, k * NCHUNK + (j + 1) * 512)
                        nc.tensor.matmul(
                            pp[:, j * 512:(j + 1) * 512], wpgt_sb, xb_sb[:, sl],
                            start=True, stop=True,
                        )
                    nc.vector.tensor_copy(
                        proj_sb.rearrange("c h w -> c (h w)")[
                            :, k * NCHUNK:(k + 1) * NCHUNK
                        ],
                        pp,
                    )
                # maxpool 2x2: stage 1 over h pairs, stage 2 over w pairs
                t1 = t1p.tile([40, H // 2, W], BF16)
                nc.vector.tensor_max(t1, proj_sb[:, 0::2, :], proj_sb[:, 1::2, :])
                g_t = gtp.tile([8, H // 2, W // 2], BF16)
                nc.vector.tensor_max(g_t, t1[32:40, :, 0::2], t1[32:40, :, 1::2])
                hpb = hbp.tile([33, M], BF16)
                nc.vector.memset(hpb[32:33, :], 1.0)
                nc.vector.tensor_max(
                    hpb[0:32, :].rearrange("c (h w) -> c h w", h=H // 2),
                    t1[0:32, :, 0::2],
                    t1[0:32, :, 1::2],
                )

                # G = w_theta^T @ g  -> [64, M]
                Gps = psG.tile([C, M], F32)
                g_flat = g_t.rearrange("c h w -> c (h w)")
                for j in range(2):
                    nc.tensor.matmul(
                        Gps[:, j * 512:(j + 1) * 512],
                        wtheta_sb, g_flat[:, j * 512:(j + 1) * 512],
                        start=True, stop=True,
                    )
                G_sb = Gp.tile([C, M], BF16)
                nc.vector.tensor_copy(G_sb, Gps)

                # hT: transpose h' [33, M] -> [128, MT/2, 2, 34] (DoubleRow layout)
                ht_ps = psH.tile([128, MT // 2, 2, 34], BF16)
                for mt in range(MT):
                    mt2, j = divmod(mt, 2)
                    nc.tensor.transpose(
                        ht_ps[:, mt2, j, 0:33],
                        hpb[:, mt * 128:(mt + 1) * 128],
                        ident33,
                    )
                hT8_sb = hTp.tile([128, MT // 2, 2, 48], FP8)
                nc.vector.tensor_copy(hT8_sb[:, :, :, 0:34], ht_ps)

            # ---- phase B: attention ----
            o_sb = osbp.tile([33, HW], F32)
            with tc.tile_pool(name="psS", bufs=2, space="PSUM") as psS, \
                 tc.tile_pool(name="psO", bufs=2, space="PSUM") as psO:
                for k in range(NCH):
                    o_ps = psO.tile([33, NCHUNK], F32)
                    for mt2 in range(MT // 2):
                        expT = expp.tile([128, 2, NCHUNK], FP8)
                        for j in range(2):
                            mt = 2 * mt2 + j
                            sT = psS.tile([128, NCHUNK], F32)
                            for jj in range(2):
                                sl = slice(
                                    k * NCHUNK + jj * 512,
                                    k * NCHUNK + (jj + 1) * 512,
                                )
                                nc.tensor.matmul(
                                    sT[:, jj * 512:(jj + 1) * 512],
                                    G_sb[:, mt * 128:(mt + 1) * 128],
                                    xb_sb[:, sl],
                                    start=True, stop=True,
                                )
                            nc.scalar.activation(
                                expT[:, j, :], sT,
                                func=mybir.ActivationFunctionType.Exp,
                            )
                        for jj in range(2):
                            nc.tensor.matmul(
                                o_ps[:, jj * 512:(jj + 1) * 512],
                                hT8_sb[:, mt2, :, 0:33],
                                expT[:, :, jj * 512:(jj + 1) * 512],
                                start=(mt2 == 0), stop=(mt2 == MT // 2 - 1),
                                perf_mode=mybir.MatmulPerfMode.DoubleRow,
                            )
                    nc.vector.tensor_copy(
                        o_sb[:, k * NCHUNK:(k + 1) * NCHUNK], o_ps
                    )
                    nc.gpsimd.dma_start(
                        out=srow_d[b, k * NCHUNK:(k + 1) * NCHUNK],
                        in_=o_sb[32:33, k * NCHUNK:(k + 1) * NCHUNK],
                    )

            # ---- recip of sumexp, broadcast to 32 partitions ----
            rs = smallp.tile([128, HW // 128], F32)
            nc.sync.dma_start(
                out=rs, in_=srow_d[b].rearrange("(p i) -> p i", p=128)
            )
            rr = smallp.tile([128, HW // 128], F32)
            nc.vector.reciprocal(rr, rs)
            nc.sync.dma_start(
                out=rd_d[b].rearrange("(p i) -> p i", p=128), in_=rr
            )
            recipB = rbp.tile([32, HW], F32)
            rd_b = rd_d[b]
            nc.sync.dma_start(
                out=recipB,
                in_=bass.AP(
                    tensor=rd_b.tensor, offset=rd_b.offset,
                    ap=[[0, 32]] + list(rd_b.ap),
                ),
            )

            # ---- output: w_o @ (o * recip) * gamma + x ----
            with tc.tile_pool(name="psW", bufs=2, space="PSUM") as psW:
                for k in range(NCH):
                    ck = slice(k * NCHUNK, (k + 1) * NCHUNK)
                    onorm = onp.tile([32, NCHUNK], BF16)
                    nc.vector.tensor_mul(onorm, o_sb[0:32, ck], recipB[:, ck])
                    wo_ps = psW.tile([C, NCHUNK], F32)
                    for j in range(2):
                        nc.tensor.matmul(
                            wo_ps[:, j * 512:(j + 1) * 512], wot_sb,
                            onorm[:, j * 512:(j + 1) * 512],
                            start=True, stop=True,
                        )
                    outc = outp.tile([C, NCHUNK], F32)
                    nc.vector.tensor_add(outc, wo_ps, x_sb[:, ck])
                    nc.gpsimd.dma_start(out=out_d[b, :, ck], in_=outc)

    if not nc.is_finalized():
        nc.finalize()
    return nc


_NC_CACHE = {}


def _run(inputs: dict, trace: bool = False):
    if "nc" not in _NC_CACHE:
        _NC_CACHE["nc"] = _build_nc()
    nc = _NC_CACHE["nc"]

    import ml_dtypes

    x = np.ascontiguousarray(inputs["x"], dtype=np.float32).reshape(B, C, HW)
    xb16 = x.astype(ml_dtypes.bfloat16)
    wpgt = np.ascontiguousarray(
        np.concatenate([inputs["w_g"], inputs["w_phi"]], axis=0).T.astype(
            ml_dtypes.bfloat16
        )
    )
    wtheta = np.ascontiguousarray(
        np.asarray(inputs["w_theta"]).astype(ml_dtypes.bfloat16)
    )
    wot = np.ascontiguousarray(
        (float(inputs["gamma"][0]) * inputs["w_o"]).T.astype(ml_dtypes.bfloat16)
    )

    in_maps = []
    for i in range(NCORES):
        in_maps.append({
            "x": np.ascontiguousarray(x[i * BPC:(i + 1) * BPC]),
            "xb": np.ascontiguousarray(xb16[i * BPC:(i + 1) * BPC]),
            "wpgt": wpgt,
            "wtheta": wtheta,
            "wot": wot,
        })

    res = run_bass_kernel_spmd(nc, in_maps, list(range(NCORES)), trace=trace)
    out = np.concatenate([r["out"] for r in res.results], axis=0)
    return out.reshape(B, C, H, W).astype(np.float32), res


def kernel(**inputs):
    out, _ = _run(inputs, trace=False)
    return out


# revision 28
# speedup vs baseline: 2.6168x; 1.1345x over previous
import sys

sys.path.insert(0, "/opt/trn_rl_repo")

import numpy as np
from contextlib import ExitStack

import concourse.bass as bass
import concourse.bacc as bacc
import concourse.tile as tile
from concourse import mybir
from concourse.bass_utils import run_bass_kernel_spmd
from concourse.masks import make_identity

B, C, H, W = 16, 64, 64, 64
HW = H * W          # 4096
M = HW // 4         # 1024
NCORES = 8
BPC = B // NCORES   # batches per core
F32 = mybir.dt.float32
BF16 = mybir.dt.bfloat16
FP8 = mybir.dt.float8e4

NCHUNK = 1024       # n-dim chunk (columns of s^T / o)
NCH = HW // NCHUNK  # 4 chunks per batch
MT = M // 128       # 8 m-tiles of 128


def _build_nc():
    nc = bacc.Bacc(None, target_bir_lowering=False)

    x_d = nc.dram_tensor("x", [BPC, C, HW], F32, kind="ExternalInput")
    xb_d = nc.dram_tensor("xb", [BPC, C, HW], BF16, kind="ExternalInput")
    wpgt_d = nc.dram_tensor("wpgt", [C, 40], BF16, kind="ExternalInput")     # [w_g; w_phi]^T
    wtheta_d = nc.dram_tensor("wtheta", [8, C], BF16, kind="ExternalInput")  # lhsT for G
    wot_d = nc.dram_tensor("wot", [32, C], BF16, kind="ExternalInput")       # (gamma*w_o)^T
    out_d = nc.dram_tensor("out", [BPC, C, HW], F32, kind="ExternalOutput")
    srow_d = nc.dram_tensor("srow", [BPC, HW], F32)   # sumexp bounce (internal)
    rd_d = nc.dram_tensor("rd", [BPC, HW], F32)       # recip bounce (internal)

    with tile.TileContext(nc) as tc, ExitStack() as ctx:
        consts = ctx.enter_context(tc.tile_pool(name="consts", bufs=1))
        wpgt_sb = consts.tile([C, 40], BF16)
        wtheta_sb = consts.tile([8, C], BF16)
        wot_sb = consts.tile([32, C], BF16)
        ident33 = consts.tile([33, 33], BF16)
        nc.sync.dma_start(out=wpgt_sb, in_=wpgt_d[:])
        nc.sync.dma_start(out=wtheta_sb, in_=wtheta_d[:])
        nc.sync.dma_start(out=wot_sb, in_=wot_d[:])
        make_identity(nc, ident33)

        # SBUF pools
        xp = ctx.enter_context(tc.tile_pool(name="xp", bufs=2))
        xbp = ctx.enter_context(tc.tile_pool(name="xbp", bufs=2))
        projp = ctx.enter_context(tc.tile_pool(name="projp", bufs=1))
        t1p = ctx.enter_context(tc.tile_pool(name="t1p", bufs=1))
        gtp = ctx.enter_context(tc.tile_pool(name="gtp", bufs=2))
        hbp = ctx.enter_context(tc.tile_pool(name="hbp", bufs=2))
        Gp = ctx.enter_context(tc.tile_pool(name="Gp", bufs=2))
        hTp = ctx.enter_context(tc.tile_pool(name="hTp", bufs=2))
        expp = ctx.enter_context(tc.tile_pool(name="expp", bufs=6))
        ocpp = ctx.enter_context(tc.tile_pool(name="ocpp", bufs=2))
        smallp = ctx.enter_context(tc.tile_pool(name="smallp", bufs=4))
        rbp = ctx.enter_context(tc.tile_pool(name="rbp", bufs=2))
        onp = ctx.enter_context(tc.tile_pool(name="onp", bufs=2))
        outp = ctx.enter_context(tc.tile_pool(name="outp", bufs=3))

        for b in range(BPC):
            x_sb = xp.tile([C, HW], F32)
            nc.sync.dma_start(out=x_sb, in_=x_d[b])
            xb_sb = xbp.tile([C, HW], BF16)
            nc.sync.dma_start(out=xb_sb, in_=xb_d[b])

            # ---- phase A: projections, pooling, G, hT ----
            with tc.tile_pool(name="psA", bufs=2, space="PSUM") as psA, \
                 tc.tile_pool(name="psG", bufs=1, space="PSUM") as psG, \
                 tc.tile_pool(name="psH", bufs=1, space="PSUM") as psH:
                proj_sb = projp.tile([40, H, W], BF16)
                for k in range(NCH):
                    pp = psA.tile([40, NCHUNK], F32)
                    for j in range(2):
                        sl = slice(k * NCHUNK + j * 512, k * NCHUNK + (j + 1) * 512)
                        nc.tensor.matmul(
                            pp[:, j * 512:(j + 1) * 512], wpgt_sb, xb_sb[:, sl],
                            start=True, stop=True,
                        )
                    nc.vector.tensor_copy(
                        proj_sb.rearrange("c h w -> c (h w)")[
                            :, k * NCHUNK:(k + 1) * NCHUNK
                        ],
                        pp,
                    )
                # maxpool 2x2: stage 1 over h pairs, stage 2 over w pairs
                t1 = t1p.tile([40, H // 2, W], BF16)
                nc.vector.tensor_max(t1, proj_sb[:, 0::2, :], proj_sb[:, 1::2, :])
                g_t = gtp.tile([8, H // 2, W // 2], BF16)
                nc.vector.tensor_max(g_t, t1[32:40, :, 0::2], t1[32:40, :, 1::2])
                hpb = hbp.tile([33, M], BF16)
                nc.vector.memset(hpb[32:33, :], 1.0)
                nc.vector.tensor_max(
                    hpb[0:32, :].rearrange("c (h w) -> c h w", h=H // 2),
                    t1[0:32, :, 0::2],
                    t1[0:32, :, 1::2],
                )

                # G = w_theta^T @ g  -> [64, M]
                Gps = psG.tile([C, M], F32)
                g_flat = g_t.rearrange("c h w -> c (h w)")
                for j in range(2):
                    nc.tensor.matmul(
                        Gps[:, j * 512:(j + 1) * 512],
                        wtheta_sb, g_flat[:, j * 512:(j + 1) * 512],
                        start=True, stop=True,
                    )
                G_sb = Gp.tile([C, M], BF16)
                nc.vector.tensor_copy(G_sb, Gps)

                # hT: transpose h' [33, M] -> [128, MT/2, 2, 34] (DoubleRow layout)
                ht_ps = psH.tile([128, MT // 2, 2, 34], BF16)
                for mt in range(MT):
                    mt2, j = divmod(mt, 2)
                    nc.tensor.transpose(
                        ht_ps[:, mt2, j, 0:33],
                        hpb[:, mt * 128:(mt + 1) * 128],
                        ident33,
                    )
                hT8_sb = hTp.tile([128, MT // 2, 2, 48], FP8)
                nc.vector.tensor_copy(hT8_sb[:, :, :, 0:34], ht_ps)

            # ---- phase B+C: attention + normalize + output, per chunk ----
            # PE emission order per chunk: s0 s1 s2 D0 s3 s4 D1 s5 s6 D2 s7
            # [wo(prev)] D3 -- keeps the in-order PE queue fed while the
            # ACT/DMA/DVE normalization chain for the previous chunk drains.
            with tc.tile_pool(name="psS", bufs=2, space="PSUM") as psS, \
                 tc.tile_pool(name="psO", bufs=1, space="PSUM") as psO, \
                 tc.tile_pool(name="psW", bufs=1, space="PSUM") as psW:
                seq = [("s", 0), ("s", 1), ("s", 2), ("D", 0), ("s", 3),
                       ("s", 4), ("D", 1), ("s", 5), ("s", 6), ("D", 2),
                       ("s", 7), ("wo", None), ("D", 3)]

                def emit_wo(pend):
                    ponorm, pck = pend
                    wo_ps = psW.tile([C, NCHUNK], F32)
                    for jj in range(2):
                        nc.tensor.matmul(
                            wo_ps[:, jj * 512:(jj + 1) * 512], wot_sb,
                            ponorm[:, jj * 512:(jj + 1) * 512],
                            start=True, stop=True,
                        )
                    outc = outp.tile([C, NCHUNK], F32)
                    nc.vector.tensor_add(outc, wo_ps, x_sb[:, pck])
                    nc.gpsimd.dma_start(out=out_d[b, :, pck], in_=outc)

                pend = None
                for k in range(NCH):
                    ck = slice(k * NCHUNK, (k + 1) * NCHUNK)
                    o_ps = psO.tile([33, NCHUNK], F32)
                    expTs = {}
                    for op, idx in seq:
                        if op == "s":
                            mt = idx
                            mt2, j = divmod(mt, 2)
                            if j == 0:
                                expTs[mt2] = expp.tile(
                                    [128, 2, NCHUNK], FP8, name=f"expT{mt2}"
                                )
                            sT = psS.tile([128, NCHUNK], F32)
                            for jj in range(2):
                                sl = slice(
                                    k * NCHUNK + jj * 512,
                                    k * NCHUNK + (jj + 1) * 512,
                                )
                                nc.tensor.matmul(
                                    sT[:, jj * 512:(jj + 1) * 512],
                                    G_sb[:, mt * 128:(mt + 1) * 128],
                                    xb_sb[:, sl], start=True, stop=True,
                                )
                            nc.scalar.activation(
                                expTs[mt2][:, j, :], sT,
                                func=mybir.ActivationFunctionType.Exp,
                            )
                        elif op == "D":
                            mt2 = idx
                            for jj in range(2):
                                nc.tensor.matmul(
                                    o_ps[:, jj * 512:(jj + 1) * 512],
                                    hT8_sb[:, mt2, :, 0:33],
                                    expTs[mt2][:, :, jj * 512:(jj + 1) * 512],
                                    start=(mt2 == 0), stop=(mt2 == MT // 2 - 1),
                                    perf_mode=mybir.MatmulPerfMode.DoubleRow,
                                )
                        elif pend is not None:
                            emit_wo(pend)
                            pend = None
                    # per-chunk normalization (off the PE critical path)
                    o_cp = ocpp.tile([33, NCHUNK], F32)
                    nc.vector.tensor_copy(o_cp, o_ps)
                    nc.sync.dma_start(out=srow_d[b, ck], in_=o_cp[32:33, :])
                    rs = smallp.tile([128, NCHUNK // 128], F32)
                    nc.sync.dma_start(
                        out=rs,
                        in_=srow_d[b, ck].rearrange("(p i) -> p i", p=128),
                    )
                    rr = smallp.tile([128, NCHUNK // 128], F32)
                    nc.vector.reciprocal(rr, rs)
                    nc.sync.dma_start(
                        out=rd_d[b, ck].rearrange("(p i) -> p i", p=128),
                        in_=rr,
                    )
                    recipB = rbp.tile([32, NCHUNK], F32)
                    rd_ck = rd_d[b, ck]
                    nc.sync.dma_start(
                        out=recipB,
                        in_=bass.AP(
                            tensor=rd_ck.tensor, offset=rd_ck.offset,
                            ap=[[0, 32]] + list(rd_ck.ap),
                        ),
                    )
                    onorm = onp.tile([32, NCHUNK], BF16)
                    nc.vector.tensor_mul(onorm, o_cp[0:32, :], recipB)
                    pend = (onorm, ck)
                emit_wo(pend)

    if not nc.is_finalized():
        nc.finalize()
    return nc


_NC_CACHE = {}


def _run(inputs: dict, trace: bool = False):
    if "nc" not in _NC_CACHE:
        _NC_CACHE["nc"] = _build_nc()
    nc = _NC_CACHE["nc"]

    import ml_dtypes

    x = np.ascontiguousarray(inputs["x"], dtype=np.float32).reshape(B, C, HW)
    xb16 = x.astype(ml_dtypes.bfloat16)
    wpgt = np.ascontiguousarray(
        np.concatenate([inputs["w_g"], inputs["w_phi"]], axis=0).T.astype(
            ml_dtypes.bfloat16
        )
    )
    wtheta = np.ascontiguousarray(
        np.asarray(inputs["w_theta"]).astype(ml_dtypes.bfloat16)
    )
    wot = np.ascontiguousarray(
        (float(inputs["gamma"][0]) * inputs["w_o"]).T.astype(ml_dtypes.bfloat16)
    )

    in_maps = []
    for i in range(NCORES):
        in_maps.append({
            "x": np.ascontiguousarray(x[i * BPC:(i + 1) * BPC]),
            "xb": np.ascontiguousarray(xb16[i * BPC:(i + 1) * BPC]),
            "wpgt": wpgt,
            "wtheta": wtheta,
            "wot": wot,
        })

    res = run_bass_kernel_spmd(nc, in_maps, list(range(NCORES)), trace=trace)
    out = np.concatenate([r["out"] for r in res.results], axis=0)
    return out.reshape(B, C, H, W).astype(np.float32), res


def kernel(**inputs):
    out, _ = _run(inputs, trace=False)
    return out
